# revision 6
# baseline (speedup 1.0000x reference)
"""ConvLSTM forecast kernel for 8x TRN2 NeuronCores (Bass/Tile).

Strategy: the model is a 3-layer ConvLSTM recurrence over 6 timesteps —
sequential in t and channel-coupled in every conv, so the only
communication-free parallel axis is the batch (m=2). Measured intra-chip
collective latency here is ~0.5 ms per hop, far above the ~40 us/step of
compute an 8-way spatial split would leave per core, so fine-grained
sharding loses. We run data-parallel over m: core 0 computes sample 0,
core 1 computes sample 1 (cores 2-7 run the same SPMD program on
duplicate data; their outputs are ignored).

Per-core kernel: each 5x5 conv = 25 shifted 1x1 convs accumulated in
PSUM (out[Cout_tile, 512 px] += Wtap[Cin_tile, Cout_tile].T @
xpad[Cin_tile, shifted 16x32 window]), bf16 operands, fp32 accumulation,
fp32 cell state. Input buffers are arranged so every h-state write is
partition-aligned and shared between layers:
  in1  [75,36,36] = h1 (parts 0-63) + x_t (parts 64-74)   (conv1 K-tile)
  inh2 [128,36,36] = h2                                    (conv2 K0, conv3 K0)
  inh3 [128,36,36] = h3                                    (conv3 K1)
  conv2 K1 = in1[0:64] (h1)  — no duplicate h copies anywhere.
"""

import sys

if "/opt/trn_rl_repo" not in sys.path:
    sys.path.insert(0, "/opt/trn_rl_repo")

import ml_dtypes
import numpy as np

import bass_rust as _bass_rust
import concourse.bass as bass
import concourse.mybir as mybir
from concourse.bass_utils import run_bass_kernel_spmd
from concourse.tile import TileContext

F32 = mybir.dt.float32
BF16 = mybir.dt.bfloat16
AF = mybir.ActivationFunctionType

H = W = 32
T = 6
IN_C = 11
HC1, HC2, HC3 = 64, 128, 128
TAPS = 25
NT = 2          # pixel tiles per conv (16 rows x 32 cols = 512 px each)
NPX = 512
N_ST = 35


def _split_excess_waits(nc, max_waits=1):
    # This walrus build rejects instructions with >1 sync wait ("Too many
    # sync wait commands"); move overflow waits onto NoOps spliced before.
    seq = 0
    for f in nc.m.functions:
        blocks = f.blocks
        try:
            items = list(blocks.items())
        except AttributeError:
            items = [(getattr(b, "name", str(i)), b) for i, b in enumerate(blocks)]
        for _, bb in items:
            out = []
            changed = False
            for inst in bb.instructions:
                si = inst.sync_info
                waits = list(si.on_wait) if si is not None and si.on_wait else []
                if len(waits) > max_waits:
                    changed = True
                    excess = waits[: len(waits) - max_waits]
                    si.on_wait = waits[len(waits) - max_waits:]
                    for i in range(0, len(excess), max_waits):
                        seq += 1
                        nop = mybir.InstNoOp(name=f"WSPL-{seq}-{inst.name}", ins=[], outs=[])
                        nop.engine = inst.engine
                        nop.sync_info = mybir.SyncInfo(
                            on_wait=excess[i : i + max_waits], on_update=[]
                        )
                        out.append(nop)
                out.append(inst)
            if changed:
                bb.instructions[:] = out


class _PatchedTileContext(TileContext):
    def __exit__(self, *args):
        r = super().__exit__(*args)
        if args[0] is None:
            _split_excess_waits(self.nc)
        return r


def _bf16(a):
    return np.ascontiguousarray(np.asarray(a, np.float32).astype(ml_dtypes.bfloat16))


def _prep_weights(W1, b1, W2, b2, W3, b3, Wm, bm, Wf1, bf1, Wf2, bf2):
    """Host-side weight relayout (see module docstring for K-tile order)."""
    W1 = np.asarray(W1, np.float32)
    W2 = np.asarray(W2, np.float32)
    W3 = np.asarray(W3, np.float32)

    # conv1: K rows = [h1 (orig in-ch 11..74), x (0..10)]; out cols per tap =
    # [i(0-63), f(64-127)] then [g, o] (so tanh(g) lands on parts 0-63 and
    # sigmoid(o) on 64-127 in the second PSUM tile).
    perm_in1 = list(range(IN_C, IN_C + HC1)) + list(range(IN_C))
    perm_out1 = (
        list(range(0, 128))                     # i, f
        + list(range(192, 256))                 # g
        + list(range(128, 192))                 # o
    )
    w1 = W1.transpose(1, 2, 3, 0)[perm_in1][:, :, :, perm_out1]  # (75,5,5,256)
    w1 = w1.reshape(IN_C + HC1, TAPS * 4 * HC1)

    # conv2: K0 = in-ch 64..191 (h2), K1 = 0..63 (h1); out = i,f,o,g natural.
    w2 = W2.transpose(1, 2, 3, 0)  # (192,5,5,512)
    w2k0 = w2[HC1:].reshape(HC2, TAPS * 4 * HC2)
    w2k1 = w2[:HC1].reshape(HC1, TAPS * 4 * HC2)

    # conv3: K0 = in-ch 0..127 (h2), K1 = 128..255 (h3).
    w3 = W3.transpose(1, 2, 3, 0)  # (256,5,5,512)
    w3k0 = w3[:HC2].reshape(HC2, TAPS * 4 * HC3)
    w3k1 = w3[HC2:].reshape(HC3, TAPS * 4 * HC3)

    # meo 1x1: (5,128,1,1) -> lhsT [128,5]
    wm = np.asarray(Wm, np.float32)[:, :, 0, 0].T.copy()

    # aqi fc1: rows are (c,dy,dx) c-major; regroup per tap j=(dy*3+dx):
    # cols j*256 + u ; lhsT tap-slice = [:, j*256+mt*128 : ...]
    Wf1 = np.asarray(Wf1, np.float32).reshape(HC3, 9, 256)
    wf1 = Wf1.transpose(0, 1, 2).reshape(HC3, 9 * 256)
    # column index should be j*256+u with row c: build explicitly
    wf1 = np.transpose(Wf1, (0, 1, 2)).reshape(HC3, 9 * 256)

    # aqi fc2: (256,6) -> [128, 12] with col = k*6+u
    Wf2 = np.asarray(Wf2, np.float32)
    wf2 = np.concatenate([Wf2[:128], Wf2[128:]], axis=1)  # [128, 12]

    b1 = np.asarray(b1, np.float32)
    b1x = np.zeros((128, 2), np.float32)
    b1x[:, 0] = b1[perm_out1[:128]]
    b1x[:, 1] = b1[perm_out1[128:]]
    b2x = np.asarray(b2, np.float32).reshape(4, HC2).T.copy()    # [128,4]
    b3x = np.asarray(b3, np.float32).reshape(4, HC3).T.copy()    # [128,4]
    bmv = np.asarray(bm, np.float32).reshape(5, 1).copy()
    bf1v = np.asarray(bf1, np.float32).reshape(2, 128).T.copy()  # [128,2]
    bf2v = np.asarray(bf2, np.float32).reshape(6, 1).copy()

    return {
        "w1": _bf16(w1), "w2k0": _bf16(w2k0), "w2k1": _bf16(w2k1),
        "w3k0": _bf16(w3k0), "w3k1": _bf16(w3k1),
        "wm": _bf16(wm), "wf1": _bf16(wf1), "wf2": _bf16(wf2),
        "b1x": b1x, "b2x": b2x, "b3x": b3x,
        "bmv": bmv, "bf1v": bf1v, "bf2v": bf2v,
    }


def _build(station_locs, n_steps=T, debug=False):
    """Build the per-core Bass program (one full sample)."""
    locs = [(int(y), int(x)) for y, x in np.asarray(station_locs)]
    nc = bass.Bass(num_devices=8)

    x_d = nc.dram_tensor("x", [T, IN_C, H, W], BF16, kind="ExternalInput")
    w1_d = nc.dram_tensor("w1", [75, TAPS * 256], BF16, kind="ExternalInput")
    w2k0_d = nc.dram_tensor("w2k0", [128, TAPS * 512], BF16, kind="ExternalInput")
    w2k1_d = nc.dram_tensor("w2k1", [64, TAPS * 512], BF16, kind="ExternalInput")
    w3k0_d = nc.dram_tensor("w3k0", [128, TAPS * 512], BF16, kind="ExternalInput")
    w3k1_d = nc.dram_tensor("w3k1", [128, TAPS * 512], BF16, kind="ExternalInput")
    wm_d = nc.dram_tensor("wm", [128, 5], BF16, kind="ExternalInput")
    wf1_d = nc.dram_tensor("wf1", [128, 9 * 256], BF16, kind="ExternalInput")
    wf2_d = nc.dram_tensor("wf2", [128, 12], BF16, kind="ExternalInput")
    b1x_d = nc.dram_tensor("b1x", [128, 2], F32, kind="ExternalInput")
    b2x_d = nc.dram_tensor("b2x", [128, 4], F32, kind="ExternalInput")
    b3x_d = nc.dram_tensor("b3x", [128, 4], F32, kind="ExternalInput")
    bmv_d = nc.dram_tensor("bmv", [5, 1], F32, kind="ExternalInput")
    bf1v_d = nc.dram_tensor("bf1v", [128, 2], F32, kind="ExternalInput")
    bf2v_d = nc.dram_tensor("bf2v", [6, 1], F32, kind="ExternalInput")

    meo_d = nc.dram_tensor("out_meo", [T, 5, 1024], F32, kind="ExternalOutput")
    aqi_d = nc.dram_tensor("out_aqi", [T, 6, N_ST], F32, kind="ExternalOutput")
    if debug:
        dbg_d = nc.dram_tensor("dbg_h", [T, 3, 128, 1024], F32, kind="ExternalOutput")

    with _PatchedTileContext(nc) as tc:
        with (
            tc.tile_pool(name="wp", bufs=1) as wp,
            tc.tile_pool(name="st", bufs=1) as st,
            tc.tile_pool(name="tp", bufs=2) as tp,
            tc.tile_pool(name="pp", bufs=8, space="PSUM") as pp,
        ):
            # ---- persistent weights/biases in SBUF ----
            w1 = wp.tile([75, TAPS * 256], BF16, tag="w1")
            w2k0 = wp.tile([128, TAPS * 512], BF16, tag="w2k0")
            w2k1 = wp.tile([64, TAPS * 512], BF16, tag="w2k1")
            w3k0 = wp.tile([128, TAPS * 512], BF16, tag="w3k0")
            w3k1 = wp.tile([128, TAPS * 512], BF16, tag="w3k1")
            wm = wp.tile([128, 5], BF16, tag="wm")
            wf1 = wp.tile([128, 9 * 256], BF16, tag="wf1")
            wf2 = wp.tile([128, 12], BF16, tag="wf2")
            b1x = wp.tile([128, 2], F32, tag="b1x")
            b2x = wp.tile([128, 4], F32, tag="b2x")
            b3x = wp.tile([128, 4], F32, tag="b3x")
            bmv = wp.tile([5, 1], F32, tag="bmv")
            bf1v = wp.tile([128, 2], F32, tag="bf1v")
            bf2v = wp.tile([6, 1], F32, tag="bf2v")
            for sb, dr in (
                (w1, w1_d), (w2k0, w2k0_d), (w2k1, w2k1_d), (w3k0, w3k0_d),
                (w3k1, w3k1_d), (wm, wm_d), (wf1, wf1_d), (wf2, wf2_d),
                (b1x, b1x_d), (b2x, b2x_d), (b3x, b3x_d), (bmv, bmv_d),
                (bf1v, bf1v_d), (bf2v, bf2v_d),
            ):
                nc.sync.dma_start(out=sb[:], in_=dr[:])

            # ---- persistent state ----
            in1 = st.tile([75, 36, 36], BF16, tag="in1")     # h1 | x
            inh2 = st.tile([128, 36, 36], BF16, tag="inh2")  # h2
            inh3 = st.tile([128, 36, 36], BF16, tag="inh3")  # h3
            c1 = st.tile([128, 1024], F32, tag="c1")         # c1 on parts 64-127
            c2 = st.tile([128, 1024], F32, tag="c2")
            c3 = st.tile([128, 1024], F32, tag="c3")
            feats = st.tile([128, N_ST * 9], BF16, tag="feats")
            nc.vector.memset(in1[:], 0.0)
            nc.vector.memset(inh2[:], 0.0)
            nc.vector.memset(inh3[:], 0.0)
            nc.vector.memset(c1[:], 0.0)
            nc.vector.memset(c2[:], 0.0)
            nc.vector.memset(c3[:], 0.0)

            # conv layer descriptors: (ktiles, Cout, wtile list, psum gate map)
            conv2_k = ((inh2, 128, w2k0), (in1, 64, w2k1))
            conv3_k = ((inh2, 128, w3k0), (inh3, 128, w3k1))

            def conv_mm(ps, ktiles, cout, nt, mt):
                """Accumulate all taps x ktiles for out-channel tile mt,
                pixel tile nt into psum tile ps."""
                r0 = nt * 16
                first = True
                for tap in range(TAPS):
                    dy, dx = tap // 5, tap % 5
                    for (buf, kc, wt) in ktiles:
                        nc.tensor.matmul(
                            out=ps[:],
                            lhsT=wt[:, tap * cout + mt * 128 : tap * cout + mt * 128 + 128]
                            if cout >= 128
                            else wt[:, tap * cout : (tap + 1) * cout],
                            rhs=buf[0:kc, r0 + dy : r0 + dy + 16, dx : dx + 32],
                            start=first,
                            stop=(tap == TAPS - 1 and buf is ktiles[-1][0]),
                        )
                        first = False

            def gates_128(ps_i, ps_f, ps_o, ps_g, cst, hbuf, bias, nt):
                """Standard LSTM gate math, C=128, everything aligned."""
                sl = (slice(None), slice(nt * NPX, (nt + 1) * NPX))
                r0 = nt * 16
                s_i = tp.tile([128, NPX], F32, tag="s_i")
                s_f = tp.tile([128, NPX], F32, tag="s_f")
                s_o = tp.tile([128, NPX], F32, tag="s_o")
                t_g = tp.tile([128, NPX], F32, tag="t_g")
                nc.scalar.activation(out=s_i[:], in_=ps_i[:], func=AF.Sigmoid, bias=bias[:, 0:1])
                nc.scalar.activation(out=s_f[:], in_=ps_f[:], func=AF.Sigmoid, bias=bias[:, 1:2])
                nc.scalar.activation(out=s_o[:], in_=ps_o[:], func=AF.Sigmoid, bias=bias[:, 2:3])
                nc.scalar.activation(out=t_g[:], in_=ps_g[:], func=AF.Tanh, bias=bias[:, 3:4])
                m1 = tp.tile([128, NPX], F32, tag="m1")
                nc.vector.tensor_mul(out=m1[:], in0=s_f[:], in1=cst[sl])
                m2 = tp.tile([128, NPX], F32, tag="m2")
                nc.vector.tensor_mul(out=m2[:], in0=s_i[:], in1=t_g[:])
                nc.vector.tensor_add(out=cst[sl], in0=m1[:], in1=m2[:])
                t_c = tp.tile([128, NPX], F32, tag="t_c")
                nc.scalar.activation(out=t_c[:], in_=cst[sl], func=AF.Tanh)
                nc.vector.tensor_mul(
                    out=hbuf[0:128, r0 + 2 : r0 + 18, 2:34], in0=s_o[:], in1=t_c[:]
                )

            def gates_l1(psA, psB, nt):
                """L1 gates: psA=[i(0-63);f(64-127)], psB=[g(0-63);o(64-127)],
                c1 lives on partitions 64-127. One cross-partition DMA shift
                for m2 and one for the h1 store."""
                sl64 = (slice(64, 128), slice(nt * NPX, (nt + 1) * NPX))
                r0 = nt * 16
                sA = tp.tile([128, NPX], F32, tag="s_i")
                nc.scalar.activation(out=sA[:], in_=psA[:], func=AF.Sigmoid, bias=b1x[:, 0:1])
                tg = tp.tile([64, NPX], F32, tag="t_g")
                nc.scalar.activation(out=tg[:], in_=psB[0:64, :], func=AF.Tanh, bias=b1x[0:64, 1:2])
                so = tp.tile([128, NPX], F32, tag="s_o")
                nc.scalar.activation(out=so[64:128, :], in_=psB[64:128, :], func=AF.Sigmoid, bias=b1x[64:128, 1:2])
                m2 = tp.tile([64, NPX], F32, tag="s_f")
                nc.vector.tensor_mul(out=m2[:], in0=sA[0:64, :], in1=tg[:])
                m2s = tp.tile([128, NPX], F32, tag="m2")
                nc.sync.dma_start(out=m2s[64:128, :], in_=m2[:])   # shift 0-63 -> 64-127
                m1 = tp.tile([128, NPX], F32, tag="m1")
                nc.vector.tensor_mul(out=m1[64:128, :], in0=sA[64:128, :], in1=c1[sl64])
                nc.vector.tensor_add(out=c1[sl64], in0=m1[64:128, :], in1=m2s[64:128, :])
                t_c = tp.tile([128, NPX], F32, tag="t_c")
                nc.scalar.activation(out=t_c[64:128, :], in_=c1[sl64], func=AF.Tanh)
                h1t = tp.tile([128, NPX], BF16, tag="h1t")
                nc.vector.tensor_mul(out=h1t[64:128, :], in0=so[64:128, :], in1=t_c[64:128, :])
                # shift 64-127 -> 0-63 and scatter into padded interior rows
                nc.sync.dma_start(
                    out=in1[0:64, r0 + 2 : r0 + 18, 2:34], in_=h1t[64:128, :]
                )

            for t in range(n_steps):
                ti = t % T
                # x_t into in1 partitions 64..74 (padded interior)
                nc.sync.dma_start(out=in1[64:75, 2:34, 2:34], in_=x_d[ti])

                # ---- layer 1 ----
                pps = {}
                for nt in range(NT):
                    for mt in range(2):  # A=[i;f], B=[g;o]
                        ps = pp.tile([128, NPX], F32, tag="ps")
                        conv_mm(ps, ((in1, 75, w1),), 256, nt, mt)
                        pps[(nt, mt)] = ps
                for nt in range(NT):
                    gates_l1(pps[(nt, 0)], pps[(nt, 1)], nt)

                # ---- layer 2 ----
                pps = {}
                for nt in range(NT):
                    for mt in range(4):
                        ps = pp.tile([128, NPX], F32, tag="ps")
                        conv_mm(ps, conv2_k, 512, nt, mt)
                        pps[(nt, mt)] = ps
                for nt in range(NT):
                    gates_128(pps[(nt, 0)], pps[(nt, 1)], pps[(nt, 2)], pps[(nt, 3)],
                              c2, inh2, b2x, nt)

                # ---- layer 3 ----
                pps = {}
                for nt in range(NT):
                    for mt in range(4):
                        ps = pp.tile([128, NPX], F32, tag="ps")
                        conv_mm(ps, conv3_k, 512, nt, mt)
                        pps[(nt, mt)] = ps
                for nt in range(NT):
                    gates_128(pps[(nt, 0)], pps[(nt, 1)], pps[(nt, 2)], pps[(nt, 3)],
                              c3, inh3, b3x, nt)

                # ---- meo head: 1x1 conv 128->5 ----
                meo_sb = tp.tile([5, 1024], F32, tag="meo_sb")
                for nt in range(NT):
                    r0 = nt * 16
                    psm = pp.tile([128, NPX], F32, tag="ps")
                    nc.tensor.matmul(
                        out=psm[0:5, :], lhsT=wm[:, 0:5],
                        rhs=inh3[0:128, r0 + 2 : r0 + 18, 2:34],
                        start=True, stop=True,
                    )
                    nc.vector.tensor_scalar_add(
                        out=meo_sb[:, nt * NPX : (nt + 1) * NPX], in0=psm[0:5, :],
                        scalar1=bmv[:],
                    )
                nc.sync.dma_start(out=meo_d[ti], in_=meo_sb[:])

                # ---- aqi head: 35 station patches -> MLP ----
                for si, (y0, x0) in enumerate(locs):
                    nc.vector.tensor_copy(
                        out=feats[:, si * 9 : si * 9 + 9],
                        in_=inh3[0:128, y0 + 2 : y0 + 5, x0 + 2 : x0 + 5],
                    )
                hid_sb = []
                for mt in range(2):
                    psh = pp.tile([128, NPX], F32, tag="ps")
                    for j in range(9):
                        nc.tensor.matmul(
                            out=psh[0:128, 0:N_ST],
                            lhsT=wf1[:, j * 256 + mt * 128 : j * 256 + mt * 128 + 128],
                            rhs=feats.rearrange("p (s j) -> p j s", j=9)[:, j, :],
                            start=(j == 0), stop=(j == 8),
                        )
                    hs = tp.tile([128, N_ST], BF16, tag=f"hid{mt}")
                    nc.scalar.activation(
                        out=hs[:], in_=psh[0:128, 0:N_ST], func=AF.Tanh,
                        bias=bf1v[:, mt : mt + 1],
                    )
                    hid_sb.append(hs)
                psa = pp.tile([128, NPX], F32, tag="ps")
                for k in range(2):
                    nc.tensor.matmul(
                        out=psa[0:6, 0:N_ST], lhsT=wf2[:, k * 6 : (k + 1) * 6],
                        rhs=hid_sb[k][:], start=(k == 0), stop=(k == 1),
                    )
                aqi_sb = tp.tile([6, N_ST], F32, tag="aqi_sb")
                nc.vector.tensor_scalar_add(out=aqi_sb[:], in0=psa[0:6, 0:N_ST],
                                            scalar1=bf2v[:])
                nc.sync.dma_start(out=aqi_d[ti], in_=aqi_sb[:])

                if debug:
                    for li, (buf, cch) in enumerate(((in1, 64), (inh2, 128), (inh3, 128))):
                        dh = tp.tile([128, 32, 32], F32, tag="dbg")
                        nc.vector.memset(dh[:], 0.0)
                        nc.vector.tensor_copy(out=dh[0:cch, :, :], in_=buf[0:cch, 2:34, 2:34])
                        nc.sync.dma_start(out=dbg_d[ti, li], in_=dh[:])

    return nc


def _make_in_maps(inputs, n_cores=8):
    X = np.asarray(inputs["X"], np.float32)
    wd = _prep_weights(
        inputs["W1"], inputs["b1"], inputs["W2"], inputs["b2"],
        inputs["W3"], inputs["b3"], inputs["Wm"], inputs["bm"],
        inputs["Wf1"], inputs["bf1"], inputs["Wf2"], inputs["bf2"],
    )
    maps = []
    for core in range(n_cores):
        s = core % X.shape[0]
        m = dict(wd)
        m["x"] = _bf16(X[s])
        maps.append(m)
    return maps


def kernel(X, W1, b1, W2, b2, W3, b3, Wm, bm, Wf1, bf1, Wf2, bf2, station_locs):
    inputs = dict(X=X, W1=W1, b1=b1, W2=W2, b2=b2, W3=W3, b3=b3, Wm=Wm, bm=bm,
                  Wf1=Wf1, bf1=bf1, Wf2=Wf2, bf2=bf2, station_locs=station_locs)
    nc = _build(station_locs)
    in_maps = _make_in_maps(inputs)
    res = run_bass_kernel_spmd(nc, in_maps, core_ids=list(range(8)))
    m = np.asarray(X).shape[0]
    aqi = np.zeros((m, T, 6 * N_ST), np.float32)
    meo = np.zeros((m, T, 5 * H * W), np.float32)
    for s in range(m):
        r = res.results[s]
        for t in range(T):
            aqi[s, t] = r["out_aqi"][t].T.reshape(-1)
            meo[s, t] = r["out_meo"][t].reshape(-1)
    return aqi, meo
